# revision 1
# baseline (speedup 1.0000x reference)
"""GPT transformer (B=2,S=1024,D=512,H=8,L=6,FF=2048,V=32000) on 8 trn2 cores.

- Sequence-parallel trunk: core c owns 256 contiguous rows of the flattened
  (B*S)=2048 token stream (cores 0-3 batch 0, cores 4-7 batch 1).
- Activations TRANSPOSED in SBUF: xT [512 feat, 256 tok] f32 residual stream.
  Weights are stationary lhsT, xT the moving rhs -> no on-chip transposes.
  LayerNorm stats via ones-matmul partition reduction + PE broadcast.
- Per layer one batch-group AllGather of (KT || V) shards.
- Softmax without max-subtraction (scores are small); denominator fused into
  the ctx PSUM tile (row 64); normalization via PE-broadcast reciprocal.
- Head: vocab-sharded 4000 cols/core after an 8-way AllGather of final x.
- bf16 matmuls, f32 PSUM/residual/LN.
"""

import numpy as np
import ml_dtypes

import concourse.bass as bass
import concourse.bacc as bacc
import concourse.mybir as mybir
import concourse.tile as tile
from concourse.bass_utils import run_bass_kernel_spmd

BF = ml_dtypes.bfloat16
N_CORES = 8
B, S, D, H, L, FF, V = 2, 1024, 512, 8, 6, 2048, 32000
HD = D // H
T = (B * S) // N_CORES   # 256
DC = D // 128            # 4
FC = FF // 128           # 16
VSH = V // N_CORES       # 4000
VT = 500
KC = 8                   # key chunks of 128 (batch-local keys = 1024)
EPS = 1e-5
AF = mybir.ActivationFunctionType
DT = mybir.dt

_cache = {}


def _build():
    nc = bacc.Bacc("TRN2", target_bir_lowering=False, debug=False,
                   num_devices=N_CORES)

    x0T = nc.dram_tensor("x0T", [D, T], DT.float32, kind="ExternalInput")
    wq = nc.dram_tensor("wq", [L, D, D], DT.bfloat16, kind="ExternalInput")
    wk = nc.dram_tensor("wk", [L, D, D], DT.bfloat16, kind="ExternalInput")
    wv = nc.dram_tensor("wv", [L, D, D], DT.bfloat16, kind="ExternalInput")
    wo = nc.dram_tensor("wo", [L, D, D], DT.bfloat16, kind="ExternalInput")
    w1 = nc.dram_tensor("w1", [L, D, FF], DT.bfloat16, kind="ExternalInput")
    w2 = nc.dram_tensor("w2", [L, FF, D], DT.bfloat16, kind="ExternalInput")
    bq = nc.dram_tensor("bq", [L, 128, DC], DT.float32, kind="ExternalInput")
    bk = nc.dram_tensor("bk", [L, 128, DC], DT.float32, kind="ExternalInput")
    bv = nc.dram_tensor("bv", [L, 1, D], DT.bfloat16, kind="ExternalInput")
    bo = nc.dram_tensor("bo", [L, 128, DC], DT.float32, kind="ExternalInput")
    b1 = nc.dram_tensor("b1", [L, 128, FC], DT.float32, kind="ExternalInput")
    b2 = nc.dram_tensor("b2", [L, 128, DC], DT.float32, kind="ExternalInput")
    l1s = nc.dram_tensor("l1s", [L, 128, DC], DT.float32, kind="ExternalInput")
    l1b = nc.dram_tensor("l1b", [L, 128, DC], DT.float32, kind="ExternalInput")
    l2s = nc.dram_tensor("l2s", [L, 128, DC], DT.float32, kind="ExternalInput")
    l2b = nc.dram_tensor("l2b", [L, 128, DC], DT.float32, kind="ExternalInput")
    lfs = nc.dram_tensor("lfs", [128, DC], DT.float32, kind="ExternalInput")
    lfb = nc.dram_tensor("lfb", [128, DC], DT.float32, kind="ExternalInput")
    wout = nc.dram_tensor("wout", [D, VSH], DT.bfloat16, kind="ExternalInput")
    bout_row = nc.dram_tensor("bout_row", [1, VSH], DT.bfloat16,
                              kind="ExternalInput")
    maskc = nc.dram_tensor("maskc", [KC, 128, T], DT.bfloat16,
                           kind="ExternalInput")
    ones_in = nc.dram_tensor("ones_in", [128, 128], DT.bfloat16,
                             kind="ExternalInput")
    outp = nc.dram_tensor("out", [2048, VSH], DT.float32, kind="ExternalOutput")

    with tile.TileContext(nc) as tc:
        with (
            tc.tile_pool(name="const", bufs=1) as constp,
            tc.tile_pool(name="gen", bufs=2) as gen,
            tc.tile_pool(name="ps", bufs=1, space="PSUM") as ps,
            tc.tile_pool(name="dram", bufs=2, space="DRAM") as dram,
        ):
            def sbt(shape, dtype, name, tag, bufs):
                return gen.tile(shape, dtype, name=name, tag=tag, bufs=bufs)

            ones_sb = constp.tile([128, 128], DT.bfloat16)
            nc.sync.dma_start(out=ones_sb[:], in_=ones_in[:, :])
            mask_sb = []
            for kc in range(KC):
                m = constp.tile([128, T], DT.bfloat16, name=f"mask{kc}")
                nc.sync.dma_start(out=m[:], in_=maskc[kc, :, :])
                mask_sb.append(m)

            xT = []
            for k in range(DC):
                t = sbt([128, T], DT.float32, f"xT{k}", "res", 9)
                nc.sync.dma_start(out=t[:], in_=x0T[128 * k:128 * (k + 1), :])
                xT.append(t)

            def cast_bf(tiles, tag="cast", bufs=6):
                out = []
                for k, t in enumerate(tiles):
                    b = sbt([128, T], DT.bfloat16, f"{tag}{k}", tag, bufs)
                    nc.vector.tensor_copy(b[:], t[:])
                    out.append(b)
                return out

            def vec_load(src, name):
                v = sbt([128, src.shape[-1]], DT.float32, name, "bvec", 10)
                nc.sync.dma_start(out=v[:], in_=src)
                return v

            def layer_norm(x_tiles, s_ap, b_ap, tag):
                xb = cast_bf(x_tiles, "lnxb", 6)
                sq = []
                for k in range(DC):
                    q = sbt([128, T], DT.bfloat16, f"{tag}sq{k}", "lnsq", 6)
                    nc.vector.tensor_mul(q[:], x_tiles[k][:], x_tiles[k][:])
                    sq.append(q)
                psum_s = ps.tile([1, T], DT.float32, name=f"{tag}ps_s",
                                 tag="small", bufs=3)
                psum_q = ps.tile([1, T], DT.float32, name=f"{tag}ps_q",
                                 tag="small", bufs=3)
                for k in range(DC):
                    nc.tensor.matmul(psum_s[:], ones_sb[:, 0:1], xb[k][:],
                                     start=(k == 0), stop=(k == DC - 1))
                for k in range(DC):
                    nc.tensor.matmul(psum_q[:], ones_sb[:, 0:1], sq[k][:],
                                     start=(k == 0), stop=(k == DC - 1))
                mean = sbt([1, T], DT.float32, f"{tag}mean", "stat", 6)
                nc.scalar.mul(mean[:], psum_s[:], 1.0 / D)
                ex2 = sbt([1, T], DT.float32, f"{tag}ex2", "stat", 6)
                nc.scalar.mul(ex2[:], psum_q[:], 1.0 / D)
                m2 = sbt([1, T], DT.float32, f"{tag}m2", "stat", 6)
                nc.vector.tensor_mul(m2[:], mean[:], mean[:])
                var = sbt([1, T], DT.float32, f"{tag}var", "stat", 6)
                nc.vector.tensor_sub(var[:], ex2[:], m2[:])
                vare = sbt([1, T], DT.float32, f"{tag}vare", "stat", 6)
                nc.vector.tensor_scalar_add(vare[:], var[:], EPS)
                std = sbt([1, T], DT.float32, f"{tag}std", "stat", 6)
                nc.scalar.activation(std[:], vare[:], AF.Sqrt)
                rstd = sbt([1, T], DT.float32, f"{tag}rstd", "stat", 6)
                nc.vector.reciprocal(rstd[:], std[:])
                mr = sbt([1, T], DT.float32, f"{tag}mr", "stat", 6)
                nc.vector.tensor_mul(mr[:], mean[:], rstd[:])
                pack = sbt([1, 2 * T], DT.bfloat16, f"{tag}pack", "statp", 4)
                nc.vector.tensor_copy(pack[:, 0:T], rstd[:])
                nc.vector.tensor_copy(pack[:, T:2 * T], mr[:])
                psum_bc = ps.tile([128, 2 * T], DT.float32, name=f"{tag}psbc",
                                  tag="small", bufs=3)
                nc.tensor.matmul(psum_bc[:], ones_sb[0:1, :], pack[:],
                                 start=True, stop=True)
                bc = sbt([128, 2 * T], DT.float32, f"{tag}bc", "lnbc", 2)
                nc.vector.tensor_copy(bc[:], psum_bc[:])
                out_tiles = []
                for k in range(DC):
                    n = sbt([128, T], DT.float32, f"{tag}n{k}", "lnn", 6)
                    # y = (x*rstd - mean*rstd)*s + b
                    nc.vector.tensor_mul(n[:], x_tiles[k][:], bc[:, 0:T])
                    n2 = sbt([128, T], DT.float32, f"{tag}n2{k}", "lnn", 6)
                    nc.vector.tensor_sub(n2[:], n[:], bc[:, T:2 * T])
                    o = sbt([128, T], DT.float32, f"{tag}o{k}", "lno", 10)
                    nc.scalar.activation(o[:], n2[:], AF.Identity,
                                         scale=s_ap[:, k:k + 1],
                                         bias=b_ap[:, k:k + 1])
                    out_tiles.append(o)
                return out_tiles

            for l in range(L):
                xbf = cast_bf(xT, "xbf", 6)

                wq_sb = [sbt([128, D], DT.bfloat16, f"wq{k}", "wq", 4)
                         for k in range(DC)]
                wk_sb = [sbt([128, D], DT.bfloat16, f"wk{k}", "wk", 4)
                         for k in range(DC)]
                wv_sb = [sbt([128, D], DT.bfloat16, f"wv{k}", "wv", 4)
                         for k in range(DC)]
                for k in range(DC):
                    nc.sync.dma_start(out=wq_sb[k][:],
                                      in_=wq[l, 128 * k:128 * (k + 1), :])
                    nc.sync.dma_start(out=wk_sb[k][:],
                                      in_=wk[l, 128 * k:128 * (k + 1), :])
                    nc.sync.dma_start(out=wv_sb[k][:],
                                      in_=wv[l, 128 * k:128 * (k + 1), :])
                bq_sb = vec_load(bq[l, :, :], "bq_sb")
                bk_sb = vec_load(bk[l, :, :], "bk_sb")
                bv_sb = sbt([1, D], DT.bfloat16, "bv_sb", "bvrow", 2)
                nc.sync.dma_start(out=bv_sb[:], in_=bv[l, :, :])

                qt, kt = [], []
                for m in range(DC):
                    pq = ps.tile([128, T], DT.float32, name=f"psq{m}",
                                 tag="mm", bufs=3)
                    for k in range(DC):
                        nc.tensor.matmul(pq[:],
                                         wq_sb[k][:, 128 * m:128 * (m + 1)],
                                         xbf[k][:], start=(k == 0),
                                         stop=(k == DC - 1))
                    q = sbt([128, T], DT.bfloat16, f"qt{m}", "qt", 5)
                    nc.scalar.activation(q[:], pq[:], AF.Identity,
                                         bias=bq_sb[:, m:m + 1])
                    qt.append(q)
                    pk = ps.tile([128, T], DT.float32, name=f"psk{m}",
                                 tag="mm", bufs=3)
                    for k in range(DC):
                        nc.tensor.matmul(pk[:],
                                         wk_sb[k][:, 128 * m:128 * (m + 1)],
                                         xbf[k][:], start=(k == 0),
                                         stop=(k == DC - 1))
                    kk = sbt([128, T], DT.bfloat16, f"kt{m}", "kt", 5)
                    nc.scalar.activation(kk[:], pk[:], AF.Identity,
                                         bias=bk_sb[:, m:m + 1])
                    kt.append(kk)
                vloc = []
                for tch in range(2):
                    pv = ps.tile([128, D], DT.float32, name=f"psvl{tch}",
                                 tag="mm", bufs=3)
                    for k in range(DC):
                        nc.tensor.matmul(
                            pv[:], xbf[k][:, 128 * tch:128 * (tch + 1)],
                            wv_sb[k][:], start=(k == 0), stop=False)
                    nc.tensor.matmul(pv[:], ones_sb[0:1, :], bv_sb[:],
                                     start=False, stop=True)
                    v = sbt([128, D], DT.bfloat16, f"vloc{tch}", "vloc", 3)
                    nc.vector.tensor_copy(v[:], pv[:])
                    vloc.append(v)

                # AllGather (KT || V) within batch group of 4 cores
                ag_in = dram.tile([1024, T], DT.bfloat16, name="ag_in",
                                  tag="ag_in", bufs=2)
                ag_out = dram.tile([4 * 1024, T], DT.bfloat16,
                                   name="ag_out", tag="ag_out", bufs=2)
                for k in range(DC):
                    nc.sync.dma_start(out=ag_in[128 * k:128 * (k + 1), :],
                                      in_=kt[k][:])
                for tch in range(2):
                    nc.sync.dma_start(
                        out=ag_in[512 + 256 * tch:512 + 256 * (tch + 1), :]
                        .rearrange("(t two) f -> t (two f)", two=2),
                        in_=vloc[tch][:])
                nc.gpsimd.collective_compute(
                    "AllGather", mybir.AluOpType.bypass,
                    replica_groups=[[0, 1, 2, 3], [4, 5, 6, 7]],
                    ins=[ag_in.opt()], outs=[ag_out.opt()],
                )

                kt_g, v_g = [], []
                for j in range(4):
                    for k in range(DC):
                        g = sbt([128, T], DT.bfloat16, f"ktg{j}_{k}",
                                "ktg", 16)
                        nc.sync.dma_start(
                            out=g[:],
                            in_=ag_out[1024 * j + 128 * k:
                                       1024 * j + 128 * (k + 1), :])
                        kt_g.append(g)
                    for tch in range(2):
                        g = sbt([128, D], DT.bfloat16, f"vg{j}_{tch}",
                                "vg", 8)
                        nc.sync.dma_start(
                            out=g[:],
                            in_=ag_out[1024 * j + 512 + 256 * tch:
                                       1024 * j + 512 + 256 * (tch + 1), :]
                            .rearrange("(t two) f -> t (two f)", two=2))
                        v_g.append(g)

                ctxT = [sbt([128, T], DT.bfloat16, f"ctxT{m}", "ctxT", 5)
                        for m in range(DC)]
                for h in range(H):
                    pctx = ps.tile([65, T], DT.float32, name=f"pctx{h}",
                                   tag="ctx", bufs=2)
                    for kc in range(KC):
                        j, half = kc // 2, kc % 2
                        lhs_k = kt_g[j * DC + h // 2][
                            64 * (h % 2):64 * (h % 2) + 64,
                            128 * half:128 * (half + 1)]
                        rhs_q = qt[h // 2][64 * (h % 2):64 * (h % 2) + 64, :]
                        psc = ps.tile([128, T], DT.float32, name=f"psc{kc}",
                                      tag="mm", bufs=3)
                        nc.tensor.matmul(psc[:], lhs_k, rhs_q,
                                         start=True, stop=True)
                        e = sbt([128, T], DT.bfloat16, f"exp{kc}", "expT", 4)
                        nc.scalar.activation(e[:], psc[:], AF.Exp,
                                             scale=0.125)
                        em = sbt([128, T], DT.bfloat16, f"expm{kc}",
                                 "expM", 4)
                        nc.vector.tensor_mul(em[:], e[:], mask_sb[kc][:])
                        nc.tensor.matmul(pctx[64:65, :], ones_sb[:, 0:1],
                                         em[:], start=(kc == 0),
                                         stop=(kc == KC - 1))
                        vtile = v_g[j * 2 + half]
                        nc.tensor.matmul(pctx[0:64, :],
                                         vtile[:, 64 * h:64 * (h + 1)],
                                         em[:], start=(kc == 0),
                                         stop=(kc == KC - 1))
                    den = sbt([1, T], DT.float32, f"den{h}", "stat", 6)
                    nc.vector.reciprocal(den[:], pctx[64:65, :])
                    denb = sbt([1, T], DT.bfloat16, f"denb{h}", "denb", 4)
                    nc.vector.tensor_copy(denb[:], den[:])
                    pbc = ps.tile([64, T], DT.float32, name=f"pbc{h}",
                                  tag="small", bufs=3)
                    nc.tensor.matmul(pbc[:], ones_sb[0:1, 0:64], denb[:],
                                     start=True, stop=True)
                    bcs = sbt([64, T], DT.float32, f"bcs{h}", "hbc", 3)
                    nc.vector.tensor_copy(bcs[:], pbc[:])
                    nc.vector.tensor_mul(
                        ctxT[h // 2][64 * (h % 2):64 * (h % 2) + 64, :],
                        pctx[0:64, :], bcs[:])

                wo_sb = [sbt([128, D], DT.bfloat16, f"wo{k}", "wo", 4)
                         for k in range(DC)]
                for k in range(DC):
                    nc.sync.dma_start(out=wo_sb[k][:],
                                      in_=wo[l, 128 * k:128 * (k + 1), :])
                bo_sb = vec_load(bo[l, :, :], "bo_sb")
                l1s_sb = vec_load(l1s[l, :, :], "l1s_sb")
                l1b_sb = vec_load(l1b[l, :, :], "l1b_sb")

                x1 = []
                for m in range(DC):
                    po = ps.tile([128, T], DT.float32, name=f"pso{m}",
                                 tag="mm", bufs=3)
                    for k in range(DC):
                        nc.tensor.matmul(po[:],
                                         wo_sb[k][:, 128 * m:128 * (m + 1)],
                                         ctxT[k][:], start=(k == 0),
                                         stop=(k == DC - 1))
                    ob = sbt([128, T], DT.float32, f"attno{m}", "epi", 4)
                    nc.scalar.activation(ob[:], po[:], AF.Identity,
                                         bias=bo_sb[:, m:m + 1])
                    xn = sbt([128, T], DT.float32, f"x1_{l}_{m}", "res", 9)
                    nc.vector.tensor_add(xn[:], ob[:], xT[m][:])
                    x1.append(xn)
                x1n = layer_norm(x1, l1s_sb, l1b_sb, f"l{l}a")

                w1_sb = [sbt([128, FF], DT.bfloat16, f"w1_{k}", "w1", 4)
                         for k in range(DC)]
                for k in range(DC):
                    nc.sync.dma_start(out=w1_sb[k][:],
                                      in_=w1[l, 128 * k:128 * (k + 1), :])
                b1_sb = sbt([128, FC], DT.float32, "b1_sb", "b1v", 2)
                nc.sync.dma_start(out=b1_sb[:], in_=b1[l, :, :])
                x1nb = cast_bf(x1n, "x1nb", 6)
                h1 = []
                for f in range(FC):
                    ph = ps.tile([128, T], DT.float32, name=f"psh{f}",
                                 tag="mm", bufs=3)
                    for k in range(DC):
                        nc.tensor.matmul(ph[:],
                                         w1_sb[k][:, 128 * f:128 * (f + 1)],
                                         x1nb[k][:], start=(k == 0),
                                         stop=(k == DC - 1))
                    hb = sbt([128, T], DT.bfloat16, f"h1_{f}", "h1", FC)
                    nc.scalar.activation(hb[:], ph[:], AF.Relu,
                                         bias=b1_sb[:, f:f + 1])
                    h1.append(hb)
                w2_sb = [sbt([128, D], DT.bfloat16, f"w2_{f}", "w2", FC)
                         for f in range(FC)]
                for f in range(FC):
                    nc.sync.dma_start(out=w2_sb[f][:],
                                      in_=w2[l, 128 * f:128 * (f + 1), :])
                b2_sb = vec_load(b2[l, :, :], "b2_sb")
                l2s_sb = vec_load(l2s[l, :, :], "l2s_sb")
                l2b_sb = vec_load(l2b[l, :, :], "l2b_sb")
                x2 = []
                for m in range(DC):
                    pf = ps.tile([128, T], DT.float32, name=f"psf{m}",
                                 tag="mm", bufs=3)
                    for f in range(FC):
                        nc.tensor.matmul(pf[:],
                                         w2_sb[f][:, 128 * m:128 * (m + 1)],
                                         h1[f][:], start=(f == 0),
                                         stop=(f == FC - 1))
                    fb = sbt([128, T], DT.float32, f"ffo{m}", "epi", 4)
                    nc.scalar.activation(fb[:], pf[:], AF.Identity,
                                         bias=b2_sb[:, m:m + 1])
                    xn = sbt([128, T], DT.float32, f"x2_{l}_{m}", "res", 9)
                    nc.vector.tensor_add(xn[:], fb[:], x1n[m][:])
                    x2.append(xn)
                xT = layer_norm(x2, l2s_sb, l2b_sb, f"l{l}b")

            lfs_sb = vec_load(lfs[:, :], "lfs_sb")
            lfb_sb = vec_load(lfb[:, :], "lfb_sb")
            xf = layer_norm(xT, lfs_sb, lfb_sb, "lnf")
            xfb = cast_bf(xf, "xfb", 6)

            agf_in = dram.tile([D, T], DT.bfloat16, name="agf_in")
            agf_out = dram.tile([N_CORES * D, T], DT.bfloat16, name="agf_out")
            for k in range(DC):
                nc.sync.dma_start(out=agf_in[128 * k:128 * (k + 1), :],
                                  in_=xfb[k][:])
            nc.gpsimd.collective_compute(
                "AllGather", mybir.AluOpType.bypass,
                replica_groups=[list(range(N_CORES))],
                ins=[agf_in.opt()], outs=[agf_out.opt()],
            )

            wout_sb = [constp.tile([128, VSH], DT.bfloat16, name=f"wout{k}")
                       for k in range(DC)]
            for k in range(DC):
                nc.sync.dma_start(out=wout_sb[k][:],
                                  in_=wout[128 * k:128 * (k + 1), :])
            bout_sb = constp.tile([1, VSH], DT.bfloat16, name="bout_sb")
            nc.sync.dma_start(out=bout_sb[:], in_=bout_row[:, :])

            for r in range(N_CORES):
                xf_r = [sbt([128, T], DT.bfloat16, f"xfr{r}_{k}", "xfr", 8)
                        for k in range(DC)]
                for k in range(DC):
                    nc.sync.dma_start(
                        out=xf_r[k][:],
                        in_=agf_out[D * r + 128 * k:D * r + 128 * (k + 1), :])
                for half in range(2):
                    trow = 256 * r + 128 * half
                    for vt in range(VSH // VT):
                        pv = ps.tile([128, VT], DT.float32,
                                     name=f"pshd{r}_{half}_{vt}",
                                     tag="mm", bufs=3)
                        for k in range(DC):
                            nc.tensor.matmul(
                                pv[:],
                                xf_r[k][:, 128 * half:128 * (half + 1)],
                                wout_sb[k][:, VT * vt:VT * (vt + 1)],
                                start=(k == 0), stop=False)
                        nc.tensor.matmul(
                            pv[:], ones_sb[0:1, :],
                            bout_sb[:, VT * vt:VT * (vt + 1)],
                            start=False, stop=True)
                        ov = sbt([128, VT], DT.float32, f"outv{vt}",
                                 "outv", 3)
                        nc.vector.tensor_copy(ov[:], pv[:])
                        nc.sync.dma_start(
                            out=outp[trow:trow + 128, VT * vt:VT * (vt + 1)],
                            in_=ov[:])

    nc.compile()
    return nc


def kernel(tokens, mask, pe, tok_emb, Wq, bq, Wk, bk, Wv, bv, Wo, bo,
           ln1_s, ln1_b, W1, b1, W2, b2, ln2_s, ln2_b,
           lnf_s, lnf_b, Wout, bout):
    if "nc" not in _cache:
        _cache["nc"] = _build()
    nc = _cache["nc"]

    tokens = np.asarray(tokens)
    x0 = (np.asarray(tok_emb)[tokens.reshape(-1)] +
          np.asarray(pe)[0][np.tile(np.arange(S), B)]).astype(np.float32)

    def bfc(a):
        return np.ascontiguousarray(np.asarray(a), dtype=BF)

    def chunkvec(a):  # [..., N] -> [..., 128, N//128]
        a = np.asarray(a, dtype=np.float32)
        lead = a.shape[:-1]
        return np.ascontiguousarray(
            a.reshape(*lead, -1, 128).swapaxes(-1, -2))

    common = dict(
        wq=bfc(Wq), wk=bfc(Wk), wv=bfc(Wv), wo=bfc(Wo),
        w1=bfc(W1), w2=bfc(W2),
        bq=chunkvec(bq), bk=chunkvec(bk),
        bv=bfc(np.asarray(bv)[:, None, :]),
        bo=chunkvec(bo), b1=chunkvec(b1), b2=chunkvec(b2),
        l1s=chunkvec(ln1_s), l1b=chunkvec(ln1_b),
        l2s=chunkvec(ln2_s), l2b=chunkvec(ln2_b),
        lfs=chunkvec(lnf_s), lfb=chunkvec(lnf_b),
        ones_in=np.ones((128, 128), dtype=BF),
    )

    mask_np = np.asarray(mask)[0, 0]
    in_maps = []
    for c in range(N_CORES):
        q0 = 256 * (c % 4)
        mc = np.zeros((KC, 128, T), dtype=BF)
        for kc in range(KC):
            mc[kc] = mask_np[q0:q0 + T, 128 * kc:128 * (kc + 1)].T.astype(BF)
        vs = slice(VSH * c, VSH * (c + 1))
        m = dict(common)
        m.update(
            x0T=np.ascontiguousarray(x0[256 * c:256 * (c + 1)].T),
            maskc=mc,
            wout=bfc(np.asarray(Wout)[:, vs]),
            bout_row=np.ascontiguousarray(
                np.asarray(bout)[vs][None, :]).astype(BF),
        )
        in_maps.append(m)

    res = run_bass_kernel_spmd(nc, in_maps, core_ids=list(range(N_CORES)))
    _cache["last_res"] = res
    out = np.concatenate([res.results[c]["out"] for c in range(N_CORES)],
                         axis=1)
    return out.reshape(B, S, V)



# revision 8
# speedup vs baseline: 1.0788x; 1.0788x over previous
"""GPT transformer (B=2,S=1024,D=512,H=8,L=6,FF=2048,V=32000) on 8 trn2 cores.

- Sequence-parallel trunk: core c owns 256 contiguous rows of the flattened
  (B*S)=2048 token stream (cores 0-3 batch 0, cores 4-7 batch 1).
- Activations TRANSPOSED in SBUF: xT [512 feat, 256 tok] f32 residual stream.
  Weights are stationary lhsT, xT the moving rhs -> no on-chip transposes.
  LayerNorm stats via ones-matmul partition reduction + PE broadcast.
- Per layer one batch-group AllGather of (KT || V) shards.
- Softmax without max-subtraction (scores are small); denominator fused into
  the ctx PSUM tile (row 64); normalization via PE-broadcast reciprocal.
- Head: vocab-sharded 4000 cols/core after an 8-way AllGather of final x.
- bf16 matmuls, f32 PSUM/residual/LN.
"""

import numpy as np
import ml_dtypes

import concourse.bass as bass
import concourse.bacc as bacc
import concourse.mybir as mybir
import concourse.tile as tile
from concourse.bass_utils import run_bass_kernel_spmd

BF = ml_dtypes.bfloat16
N_CORES = 8
B, S, D, H, L, FF, V = 2, 1024, 512, 8, 6, 2048, 32000
HD = D // H
T = (B * S) // N_CORES   # 256
DC = D // 128            # 4
FC = FF // 128           # 16
VSH = V // N_CORES       # 4000
VT = 500
KC = 8                   # key chunks of 128 (batch-local keys = 1024)
EPS = 1e-5
AF = mybir.ActivationFunctionType
DT = mybir.dt

_cache = {}


def _build():
    nc = bacc.Bacc("TRN2", target_bir_lowering=False, debug=False,
                   num_devices=N_CORES)

    x0T = nc.dram_tensor("x0T", [D, T], DT.float32, kind="ExternalInput")
    wq = nc.dram_tensor("wq", [L, D, D], DT.bfloat16, kind="ExternalInput")
    wk = nc.dram_tensor("wk", [L, D, D], DT.bfloat16, kind="ExternalInput")
    wv = nc.dram_tensor("wv", [L, D, D], DT.bfloat16, kind="ExternalInput")
    wo = nc.dram_tensor("wo", [L, D, D], DT.bfloat16, kind="ExternalInput")
    w1 = nc.dram_tensor("w1", [L, D, FF], DT.bfloat16, kind="ExternalInput")
    w2 = nc.dram_tensor("w2", [L, FF, D], DT.bfloat16, kind="ExternalInput")
    bq = nc.dram_tensor("bq", [L, 128, DC], DT.float32, kind="ExternalInput")
    bk = nc.dram_tensor("bk", [L, 128, DC], DT.float32, kind="ExternalInput")
    bv = nc.dram_tensor("bv", [L, 1, D], DT.bfloat16, kind="ExternalInput")
    bo = nc.dram_tensor("bo", [L, 128, DC], DT.float32, kind="ExternalInput")
    b1 = nc.dram_tensor("b1", [L, 128, FC], DT.float32, kind="ExternalInput")
    b2 = nc.dram_tensor("b2", [L, 128, DC], DT.float32, kind="ExternalInput")
    l1s = nc.dram_tensor("l1s", [L, 128, DC], DT.float32, kind="ExternalInput")
    l1b = nc.dram_tensor("l1b", [L, 128, DC], DT.float32, kind="ExternalInput")
    l2s = nc.dram_tensor("l2s", [L, 128, DC], DT.float32, kind="ExternalInput")
    l2b = nc.dram_tensor("l2b", [L, 128, DC], DT.float32, kind="ExternalInput")
    lfs = nc.dram_tensor("lfs", [128, DC], DT.float32, kind="ExternalInput")
    lfb = nc.dram_tensor("lfb", [128, DC], DT.float32, kind="ExternalInput")
    wout = nc.dram_tensor("wout", [D, VSH], DT.bfloat16, kind="ExternalInput")
    bout_row = nc.dram_tensor("bout_row", [1, VSH], DT.bfloat16,
                              kind="ExternalInput")
    maskc = nc.dram_tensor("maskc", [KC, 128, T], DT.bfloat16,
                           kind="ExternalInput")
    ones_in = nc.dram_tensor("ones_in", [128, 128], DT.bfloat16,
                             kind="ExternalInput")
    outp = nc.dram_tensor("out", [2048 + T, VSH], DT.bfloat16,
                          kind="ExternalOutput")

    with tile.TileContext(nc) as tc:
        with (
            tc.tile_pool(name="const", bufs=1) as constp,
            tc.tile_pool(name="gen", bufs=2) as gen,
            tc.tile_pool(name="ps", bufs=1, space="PSUM") as ps,
            tc.tile_pool(name="dram", bufs=2, space="DRAM") as dram,
        ):
            def sbt(shape, dtype, name, tag, bufs):
                return gen.tile(shape, dtype, name=name, tag=tag, bufs=bufs)

            ones_sb = constp.tile([128, 128], DT.bfloat16)
            nc.sync.dma_start(out=ones_sb[:], in_=ones_in[:, :])
            mask_sb = []
            for kc in range(KC):
                m = constp.tile([128, T], DT.bfloat16, name=f"mask{kc}")
                nc.sync.dma_start(out=m[:], in_=maskc[kc, :, :])
                mask_sb.append(m)

            xT = []
            for k in range(DC):
                t = sbt([128, T], DT.float32, f"xT{k}", "res", 9)
                nc.sync.dma_start(out=t[:], in_=x0T[128 * k:128 * (k + 1), :])
                xT.append(t)

            def cast_bf(tiles, tag="cast", bufs=6):
                out = []
                for k, t in enumerate(tiles):
                    b = sbt([128, T], DT.bfloat16, f"{tag}{k}", tag, bufs)
                    nc.vector.tensor_copy(b[:], t[:])
                    out.append(b)
                return out

            def vec_load(src, name):
                v = sbt([128, src.shape[-1]], DT.float32, name, "bvec", 10)
                nc.sync.dma_start(out=v[:], in_=src)
                return v

            def layer_norm(x_tiles, s_ap, b_ap, tag):
                xb = cast_bf(x_tiles, "lnxb", 6)
                sq = []
                for k in range(DC):
                    q = sbt([128, T], DT.bfloat16, f"{tag}sq{k}", "lnsq", 6)
                    nc.vector.tensor_mul(q[:], x_tiles[k][:], x_tiles[k][:])
                    sq.append(q)
                psum_s = ps.tile([1, T], DT.float32, name=f"{tag}ps_s",
                                 tag="small", bufs=3)
                psum_q = ps.tile([1, T], DT.float32, name=f"{tag}ps_q",
                                 tag="small", bufs=3)
                for k in range(DC):
                    nc.tensor.matmul(psum_s[:], ones_sb[:, 0:1], xb[k][:],
                                     start=(k == 0), stop=(k == DC - 1))
                for k in range(DC):
                    nc.tensor.matmul(psum_q[:], ones_sb[:, 0:1], sq[k][:],
                                     start=(k == 0), stop=(k == DC - 1))
                mean = sbt([1, T], DT.float32, f"{tag}mean", "stat", 6)
                nc.scalar.mul(mean[:], psum_s[:], 1.0 / D)
                ex2 = sbt([1, T], DT.float32, f"{tag}ex2", "stat", 6)
                nc.scalar.mul(ex2[:], psum_q[:], 1.0 / D)
                m2 = sbt([1, T], DT.float32, f"{tag}m2", "stat", 6)
                nc.vector.tensor_mul(m2[:], mean[:], mean[:])
                var = sbt([1, T], DT.float32, f"{tag}var", "stat", 6)
                nc.vector.tensor_sub(var[:], ex2[:], m2[:])
                vare = sbt([1, T], DT.float32, f"{tag}vare", "stat", 6)
                nc.vector.tensor_scalar_add(vare[:], var[:], EPS)
                std = sbt([1, T], DT.float32, f"{tag}std", "stat", 6)
                nc.scalar.activation(std[:], vare[:], AF.Sqrt)
                rstd = sbt([1, T], DT.float32, f"{tag}rstd", "stat", 6)
                nc.vector.reciprocal_approx_fast(rstd[:], std[:])
                mr = sbt([1, T], DT.float32, f"{tag}mr", "stat", 6)
                nc.vector.tensor_mul(mr[:], mean[:], rstd[:])
                pack = sbt([1, 2 * T], DT.bfloat16, f"{tag}pack", "statp", 4)
                nc.vector.tensor_copy(pack[:, 0:T], rstd[:])
                nc.vector.tensor_copy(pack[:, T:2 * T], mr[:])
                psum_bc = ps.tile([128, 2 * T], DT.float32, name=f"{tag}psbc",
                                  tag="small", bufs=3)
                nc.tensor.matmul(psum_bc[:], ones_sb[0:1, :], pack[:],
                                 start=True, stop=True)
                bc = sbt([128, 2 * T], DT.float32, f"{tag}bc", "lnbc", 2)
                nc.vector.tensor_copy(bc[:], psum_bc[:])
                out_tiles = []
                for k in range(DC):
                    n = sbt([128, T], DT.float32, f"{tag}n{k}", "lnn", 4)
                    # y = (x*rstd - mean*rstd)*s + b
                    nc.vector.tensor_mul(n[:], x_tiles[k][:], bc[:, 0:T])
                    n2 = sbt([128, T], DT.float32, f"{tag}n2{k}", "lnn", 4)
                    nc.vector.tensor_sub(n2[:], n[:], bc[:, T:2 * T])
                    o = sbt([128, T], DT.float32, f"{tag}o{k}", "lno", 8)
                    nc.scalar.activation(o[:], n2[:], AF.Identity,
                                         scale=s_ap[:, k:k + 1],
                                         bias=b_ap[:, k:k + 1])
                    out_tiles.append(o)
                return out_tiles

            for l in range(L):
                xbf = cast_bf(xT, "xbf", 6)

                wq_sb = [sbt([128, D], DT.bfloat16, f"wq{k}", "wq", 4)
                         for k in range(DC)]
                wk_sb = [sbt([128, D], DT.bfloat16, f"wk{k}", "wk", 4)
                         for k in range(DC)]
                wv_sb = [sbt([128, D], DT.bfloat16, f"wv{k}", "wv", 4)
                         for k in range(DC)]
                for k in range(DC):
                    nc.sync.dma_start(out=wk_sb[k][:],
                                      in_=wk[l, 128 * k:128 * (k + 1), :])
                    nc.sync.dma_start(out=wv_sb[k][:],
                                      in_=wv[l, 128 * k:128 * (k + 1), :])
                    nc.sync.dma_start(out=wq_sb[k][:],
                                      in_=wq[l, 128 * k:128 * (k + 1), :])
                bq_sb = vec_load(bq[l, :, :], "bq_sb")
                bk_sb = vec_load(bk[l, :, :], "bk_sb")
                bv_sb = sbt([1, D], DT.bfloat16, "bv_sb", "bvrow", 2)
                nc.sync.dma_start(out=bv_sb[:], in_=bv[l, :, :])

                # K and V first so the AllGather launches ASAP; Q after.
                kt = []
                for m in range(DC):
                    pk = ps.tile([128, T], DT.float32, name=f"psk{m}",
                                 tag="mm", bufs=3)
                    for k in range(DC):
                        nc.tensor.matmul(pk[:],
                                         wk_sb[k][:, 128 * m:128 * (m + 1)],
                                         xbf[k][:], start=(k == 0),
                                         stop=(k == DC - 1))
                    kk = sbt([128, T], DT.bfloat16, f"kt{m}", "kt", 5)
                    nc.scalar.activation(kk[:], pk[:], AF.Identity,
                                         bias=bk_sb[:, m:m + 1])
                    kt.append(kk)
                vloc = []
                for tch in range(2):
                    pv = ps.tile([128, D], DT.float32, name=f"psvl{tch}",
                                 tag="mm", bufs=3)
                    for k in range(DC):
                        nc.tensor.matmul(
                            pv[:], xbf[k][:, 128 * tch:128 * (tch + 1)],
                            wv_sb[k][:], start=(k == 0), stop=False)
                    nc.tensor.matmul(pv[:], ones_sb[0:1, :], bv_sb[:],
                                     start=False, stop=True)
                    v = sbt([128, D], DT.bfloat16, f"vloc{tch}", "vloc", 3)
                    nc.vector.tensor_copy(v[:], pv[:])
                    vloc.append(v)

                # AllGather (KT || V) within batch group of 4 cores
                ag_in = dram.tile([1024, T], DT.bfloat16, name="ag_in",
                                  tag="ag_in", bufs=2)
                ag_out = dram.tile([4 * 1024, T], DT.bfloat16,
                                   name="ag_out", tag="ag_out", bufs=2)
                for k in range(DC):
                    nc.sync.dma_start(out=ag_in[128 * k:128 * (k + 1), :],
                                      in_=kt[k][:])
                for tch in range(2):
                    nc.sync.dma_start(
                        out=ag_in[512 + 256 * tch:512 + 256 * (tch + 1), :]
                        .rearrange("(t two) f -> t (two f)", two=2),
                        in_=vloc[tch][:])
                nc.gpsimd.collective_compute(
                    "AllGather", mybir.AluOpType.bypass,
                    replica_groups=[[0, 1, 2, 3], [4, 5, 6, 7]],
                    ins=[ag_in.opt()], outs=[ag_out.opt()],
                )

                # Q while the AllGather is in flight
                qt = []
                for m in range(DC):
                    pq = ps.tile([128, T], DT.float32, name=f"psq{m}",
                                 tag="mm", bufs=3)
                    for k in range(DC):
                        nc.tensor.matmul(pq[:],
                                         wq_sb[k][:, 128 * m:128 * (m + 1)],
                                         xbf[k][:], start=(k == 0),
                                         stop=(k == DC - 1))
                    q = sbt([128, T], DT.bfloat16, f"qt{m}", "qt", 5)
                    nc.scalar.activation(q[:], pq[:], AF.Identity,
                                         bias=bq_sb[:, m:m + 1])
                    qt.append(q)

                # Gathered K tiles; V tiles land strided into [128, 8*65]
                # with a ones column per head (fused softmax denominator).
                kt_g, v_g = [], []
                for j in range(4):
                    for k in range(DC):
                        g = sbt([128, T], DT.bfloat16, f"ktg{j}_{k}",
                                "ktg", 16)
                        nc.sync.dma_start(
                            out=g[:],
                            in_=ag_out[1024 * j + 128 * k:
                                       1024 * j + 128 * (k + 1), :])
                        kt_g.append(g)
                    for tch in range(2):
                        g = sbt([128, H * 65], DT.bfloat16, f"vg{j}_{tch}",
                                "vg", 8)
                        nc.sync.dma_start(
                            out=g[:].rearrange("p (h c) -> p h c", c=65)
                            [:, :, 0:64],
                            in_=ag_out[1024 * j + 512 + 256 * tch:
                                       1024 * j + 512 + 256 * (tch + 1), :]
                            .rearrange("(t two) f -> t (two f)", two=2)
                            .rearrange("p (h c) -> p h c", c=64))
                        nc.vector.memset(
                            g[:].rearrange("p (h c) -> p h c", c=65)
                            [:, :, 64:65], 1.0)
                        v_g.append(g)

                ctxT = [sbt([128, T], DT.bfloat16, f"ctxT{m}", "ctxT", 5)
                        for m in range(DC)]
                for h in range(H):
                    pctx = ps.tile([65, T], DT.float32, name=f"pctx{h}",
                                   tag="ctx", bufs=2)
                    for kc in range(KC):
                        j, half = kc // 2, kc % 2
                        lhs_k = kt_g[j * DC + h // 2][
                            64 * (h % 2):64 * (h % 2) + 64,
                            128 * half:128 * (half + 1)]
                        rhs_q = qt[h // 2][64 * (h % 2):64 * (h % 2) + 64, :]
                        psc = ps.tile([128, T], DT.float32, name=f"psc{kc}",
                                      tag="mm", bufs=3)
                        nc.tensor.matmul(psc[:], lhs_k, rhs_q,
                                         start=True, stop=True)
                        e = sbt([128, T], DT.bfloat16, f"exp{kc}", "expT", 3)
                        nc.scalar.activation(e[:], psc[:], AF.Exp,
                                             scale=0.125)
                        em = sbt([128, T], DT.bfloat16, f"expm{kc}",
                                 "expM", 3)
                        nc.vector.tensor_mul(em[:], e[:], mask_sb[kc][:])
                        # [V | 1] stationary: rows 0-63 ctx, row 64 denom
                        vtile = v_g[j * 2 + half]
                        nc.tensor.matmul(pctx[:, :],
                                         vtile[:, 65 * h:65 * (h + 1)],
                                         em[:], start=(kc == 0),
                                         stop=(kc == KC - 1))
                    den = sbt([1, T], DT.float32, f"den{h}", "stat", 6)
                    nc.vector.tensor_copy(den[:], pctx[64:65, :])
                    rden = sbt([1, T], DT.float32, f"rden{h}", "stat", 6)
                    nc.vector.reciprocal_approx_fast(rden[:], den[:])
                    denb = sbt([1, T], DT.bfloat16, f"denb{h}", "denb", 4)
                    nc.vector.tensor_copy(denb[:], rden[:])
                    pbc = ps.tile([64, T], DT.float32, name=f"pbc{h}",
                                  tag="small", bufs=3)
                    nc.tensor.matmul(pbc[:], ones_sb[0:1, 0:64], denb[:],
                                     start=True, stop=True)
                    bcs = sbt([64, T], DT.float32, f"bcs{h}", "hbc", 3)
                    nc.vector.tensor_copy(bcs[:], pbc[:])
                    nc.vector.tensor_mul(
                        ctxT[h // 2][64 * (h % 2):64 * (h % 2) + 64, :],
                        pctx[0:64, :], bcs[:])

                wo_sb = [sbt([128, D], DT.bfloat16, f"wo{k}", "wo", 4)
                         for k in range(DC)]
                for k in range(DC):
                    nc.sync.dma_start(out=wo_sb[k][:],
                                      in_=wo[l, 128 * k:128 * (k + 1), :])
                bo_sb = vec_load(bo[l, :, :], "bo_sb")
                l1s_sb = vec_load(l1s[l, :, :], "l1s_sb")
                l1b_sb = vec_load(l1b[l, :, :], "l1b_sb")

                x1 = []
                for m in range(DC):
                    po = ps.tile([128, T], DT.float32, name=f"pso{m}",
                                 tag="mm", bufs=3)
                    for k in range(DC):
                        nc.tensor.matmul(po[:],
                                         wo_sb[k][:, 128 * m:128 * (m + 1)],
                                         ctxT[k][:], start=(k == 0),
                                         stop=(k == DC - 1))
                    ob = sbt([128, T], DT.float32, f"attno{m}", "epi", 4)
                    nc.scalar.activation(ob[:], po[:], AF.Identity,
                                         bias=bo_sb[:, m:m + 1])
                    xn = sbt([128, T], DT.float32, f"x1_{l}_{m}", "res", 9)
                    nc.vector.tensor_add(xn[:], ob[:], xT[m][:])
                    x1.append(xn)
                x1n = layer_norm(x1, l1s_sb, l1b_sb, f"l{l}a")

                w1_sb = [sbt([128, FF], DT.bfloat16, f"w1_{k}", "w1", 4)
                         for k in range(DC)]
                for k in range(DC):
                    nc.sync.dma_start(out=w1_sb[k][:],
                                      in_=w1[l, 128 * k:128 * (k + 1), :])
                b1_sb = sbt([128, FC], DT.float32, "b1_sb", "b1v", 2)
                nc.sync.dma_start(out=b1_sb[:], in_=b1[l, :, :])
                x1nb = cast_bf(x1n, "x1nb", 6)
                h1 = []
                for f in range(FC):
                    ph = ps.tile([128, T], DT.float32, name=f"psh{f}",
                                 tag="mm", bufs=3)
                    for k in range(DC):
                        nc.tensor.matmul(ph[:],
                                         w1_sb[k][:, 128 * f:128 * (f + 1)],
                                         x1nb[k][:], start=(k == 0),
                                         stop=(k == DC - 1))
                    hb = sbt([128, T], DT.bfloat16, f"h1_{f}", "h1", FC)
                    nc.scalar.activation(hb[:], ph[:], AF.Relu,
                                         bias=b1_sb[:, f:f + 1])
                    h1.append(hb)
                w2_sb = [sbt([128, D], DT.bfloat16, f"w2_{f}", "w2", FC)
                         for f in range(FC)]
                for f in range(FC):
                    nc.sync.dma_start(out=w2_sb[f][:],
                                      in_=w2[l, 128 * f:128 * (f + 1), :])
                b2_sb = vec_load(b2[l, :, :], "b2_sb")
                l2s_sb = vec_load(l2s[l, :, :], "l2s_sb")
                l2b_sb = vec_load(l2b[l, :, :], "l2b_sb")
                x2 = []
                for m in range(DC):
                    pf = ps.tile([128, T], DT.float32, name=f"psf{m}",
                                 tag="mm", bufs=3)
                    for f in range(FC):
                        nc.tensor.matmul(pf[:],
                                         w2_sb[f][:, 128 * m:128 * (m + 1)],
                                         h1[f][:], start=(f == 0),
                                         stop=(f == FC - 1))
                    fb = sbt([128, T], DT.float32, f"ffo{m}", "epi", 4)
                    nc.scalar.activation(fb[:], pf[:], AF.Identity,
                                         bias=b2_sb[:, m:m + 1])
                    xn = sbt([128, T], DT.float32, f"x2_{l}_{m}", "res", 9)
                    nc.vector.tensor_add(xn[:], fb[:], x1n[m][:])
                    x2.append(xn)
                xT = layer_norm(x2, l2s_sb, l2b_sb, f"l{l}b")

            lfs_sb = vec_load(lfs[:, :], "lfs_sb")
            lfb_sb = vec_load(lfb[:, :], "lfb_sb")
            xf = layer_norm(xT, lfs_sb, lfb_sb, "lnf")
            xfb = cast_bf(xf, "xfb", 6)

            agf_in = dram.tile([D, T], DT.bfloat16, name="agf_in")
            agf_out = dram.tile([N_CORES * D, T], DT.bfloat16,
                                name="agf_out", addr_space="Shared")
            for k in range(DC):
                nc.sync.dma_start(out=agf_in[128 * k:128 * (k + 1), :],
                                  in_=xfb[k][:])
            nc.gpsimd.collective_compute(
                "AllGather", mybir.AluOpType.bypass,
                replica_groups=[list(range(N_CORES))],
                ins=[agf_in.opt()], outs=[agf_out.opt()],
            )

            wout_sb = [constp.tile([128, VSH], DT.bfloat16, name=f"wout{k}")
                       for k in range(DC)]
            for k in range(DC):
                nc.sync.dma_start(out=wout_sb[k][:],
                                  in_=wout[128 * k:128 * (k + 1), :])
            bout_sb = constp.tile([1, VSH], DT.bfloat16, name="bout_sb")
            nc.sync.dma_start(out=bout_sb[:], in_=bout_row[:, :])
            # bias broadcast [128, VT] per vt chunk (built once via PE)
            bias_bc = []
            for vt in range(VSH // VT):
                pb = ps.tile([128, VT], DT.float32, name=f"pbb{vt}",
                             tag="small", bufs=3)
                nc.tensor.matmul(pb[:], ones_sb[0:1, :],
                                 bout_sb[:, VT * vt:VT * (vt + 1)],
                                 start=True, stop=True)
                bb = constp.tile([128, VT], DT.bfloat16, name=f"biasbc{vt}")
                nc.vector.tensor_copy(bb[:], pb[:])
                bias_bc.append(bb)

            def head_rows(x_tiles, trow):
                for half in range(2):
                    for vt in range(VSH // VT):
                        pv = ps.tile([128, VT], DT.float32,
                                     name=f"pshd{trow}_{half}_{vt}",
                                     tag="mm", bufs=3)
                        for k in range(DC):
                            nc.tensor.matmul(
                                pv[:],
                                x_tiles[k][:, 128 * half:128 * (half + 1)],
                                wout_sb[k][:, VT * vt:VT * (vt + 1)],
                                start=(k == 0), stop=(k == DC - 1))
                        ov = sbt([128, VT], DT.bfloat16, f"outv{vt}",
                                 "outv", 4)
                        nc.vector.tensor_add(ov[:], pv[:], bias_bc[vt][:])
                        nc.sync.dma_start(
                            out=outp[trow + 128 * half:trow + 128 * half
                                     + 128, VT * vt:VT * (vt + 1)],
                            in_=ov[:])

            # Own 256 rows first, straight from local xfb — overlaps the
            # 8-way AllGather. (Also recomputed in the r-loop; host uses
            # the r-loop copy, this block exists to hide AG latency.)
            head_rows(xfb, 0)

            for r in range(N_CORES):
                xf_r = [sbt([128, T], DT.bfloat16, f"xfr{r}_{k}", "xfr", 8)
                        for k in range(DC)]
                for k in range(DC):
                    nc.sync.dma_start(
                        out=xf_r[k][:],
                        in_=agf_out[D * r + 128 * k:D * r + 128 * (k + 1), :])
                head_rows(xf_r, T + 256 * r)

    nc.compile()
    return nc


def kernel(tokens, mask, pe, tok_emb, Wq, bq, Wk, bk, Wv, bv, Wo, bo,
           ln1_s, ln1_b, W1, b1, W2, b2, ln2_s, ln2_b,
           lnf_s, lnf_b, Wout, bout):
    if "nc" not in _cache:
        _cache["nc"] = _build()
    nc = _cache["nc"]

    tokens = np.asarray(tokens)
    x0 = (np.asarray(tok_emb)[tokens.reshape(-1)] +
          np.asarray(pe)[0][np.tile(np.arange(S), B)]).astype(np.float32)

    def bfc(a):
        return np.ascontiguousarray(np.asarray(a), dtype=BF)

    def chunkvec(a):  # [..., N] -> [..., 128, N//128]
        a = np.asarray(a, dtype=np.float32)
        lead = a.shape[:-1]
        return np.ascontiguousarray(
            a.reshape(*lead, -1, 128).swapaxes(-1, -2))

    common = dict(
        wq=bfc(Wq), wk=bfc(Wk), wv=bfc(Wv), wo=bfc(Wo),
        w1=bfc(W1), w2=bfc(W2),
        bq=chunkvec(bq), bk=chunkvec(bk),
        bv=bfc(np.asarray(bv)[:, None, :]),
        bo=chunkvec(bo), b1=chunkvec(b1), b2=chunkvec(b2),
        l1s=chunkvec(ln1_s), l1b=chunkvec(ln1_b),
        l2s=chunkvec(ln2_s), l2b=chunkvec(ln2_b),
        lfs=chunkvec(lnf_s), lfb=chunkvec(lnf_b),
        ones_in=np.ones((128, 128), dtype=BF),
    )

    mask_np = np.asarray(mask)[0, 0]
    in_maps = []
    for c in range(N_CORES):
        q0 = 256 * (c % 4)
        mc = np.zeros((KC, 128, T), dtype=BF)
        for kc in range(KC):
            mc[kc] = mask_np[q0:q0 + T, 128 * kc:128 * (kc + 1)].T.astype(BF)
        vs = slice(VSH * c, VSH * (c + 1))
        m = dict(common)
        m.update(
            x0T=np.ascontiguousarray(x0[256 * c:256 * (c + 1)].T),
            maskc=mc,
            wout=bfc(np.asarray(Wout)[:, vs]),
            bout_row=np.ascontiguousarray(
                np.asarray(bout)[vs][None, :]).astype(BF),
        )
        in_maps.append(m)

    res = run_bass_kernel_spmd(nc, in_maps, core_ids=list(range(N_CORES)))
    _cache["last_res"] = res
    out = np.concatenate(
        [res.results[c]["out"][T:].astype(np.float32)
         for c in range(N_CORES)], axis=1)
    return out.reshape(B, S, V)

